# revision 1
# baseline (speedup 1.0000x reference)
"""PoseConsistencyLoss Trainium2 kernel (8-core SPMD Bass/Tile).

Math: the reference's outputs (loss, num_matches, mean_distance) depend only on
the per-landmark min squared distance over all splats:
  - matched = splat_positions[argmin] makes sum(sqerr) == min_dist^2 exactly,
  - so loss = sum(valid*minsq)/max(3*num,1), mean = sum(valid*sqrt(minsq))/max(num,1),
    num = sum(minsq < 1.0).
Sharding: splats split across 8 cores (8192 each); each core computes partial
column-mins of the [8192 x 2048] distance matrix, AllGather + local min, then a
replicated masked reduction. Output taken from core 0.

Distance matrix via a K=9 feature matmul on the PE:
  L = [-2*cx,-2*cy,-2*cz, cx^2,cy^2,cz^2, 1,1,1]  (landmarks, camera frame)
  S = [ sx,  sy,  sz,  1,  1,  1,  sx^2,sy^2,sz^2] (splats)
  D2[m,n] = sum_k L[k,m]*S[k,n]
fp32 matmul is 4 cy/row on TRN2; fp32r is 1 cy/row but rounds operands to 12-bit
mantissa. We recover ~fp32 accuracy with a hi/lo split (2 matmuls):
  D2 = L_hi*S_hi  +  (L_lo*S_hi + L_hi*S_lo)   [K=9 and K=18, PSUM-accumulated]
where hi = bitwise-truncate to 11 explicit mantissa bits (exact in fp32r).

Column-min consumes PSUM with a DVE/ACT split: some spans reduced directly on
the DVE (fp32 reduce_min), others copied PSUM->SBUF by the Scalar engine and
paired into tensor_tensor_reduce(min,min) ops that consume 2 streams/cycle.
"""

import os
import sys
import time

sys.path.insert(0, "/opt/trn_rl_repo")

import numpy as np

import concourse.bass as bass
import concourse.bacc as bacc
import concourse.tile as tile
from concourse import mybir
from concourse.bass_utils import run_bass_kernel_spmd

# Disk-cache NEFF compiles (neuronxcc is ~15 min/invocation on this 1-CPU box).
import concourse.bass_utils as _bu
import concourse.bass2jax as _b2j

_orig_compile_bir = _bu.compile_bir_kernel
_NEFF_CACHE = os.environ.get("BASS_NEFF_CACHE_DIR", "/tmp/bass_neff_cache")


def _cached_compile_bir(bir_json, tmpdir, neff_name="file.neff"):
    import hashlib
    import shutil

    h = hashlib.sha256(bir_json).hexdigest()[:24]
    os.makedirs(_NEFF_CACHE, exist_ok=True)
    cpath = os.path.join(_NEFF_CACHE, f"{h}_{neff_name}")
    out = os.path.join(tmpdir, neff_name)
    if os.path.exists(cpath):
        shutil.copyfile(cpath, out)
        return out
    p = _orig_compile_bir(bir_json, tmpdir, neff_name=neff_name)
    try:
        shutil.copyfile(p, cpath)
    except OSError:
        pass
    return p


_bu.compile_bir_kernel = _cached_compile_bir
_b2j.compile_bir_kernel = _cached_compile_bir

F32 = mybir.dt.float32
F32R = mybir.dt.float32r
U32 = mybir.dt.uint32
I32 = mybir.dt.int32
AF = mybir.ActivationFunctionType
ALU = mybir.AluOpType
AX = mybir.AxisListType

HI_MASK = 0xFFFFF000  # keep sign+exp+11 mantissa bits (fp32r-exact)
BIG = 3.0e38

FULL_CFG = dict(
    n_cores=8,
    s_per_core=8192,   # splats per core
    m_total=2048,      # landmarks
    span=1024,         # psum span (free elems, 2 banks)
    # per-mt span roles, cycled: D=direct DVE reduce, A=ACT copy to SBUF,
    # T=tensor_tensor_reduce pairing the previous A's copy with its own psum.
    roles=("D",),          # direct DVE reduce only (safest instruction mix)
    use_f32r_split=False,  # False -> plain fp32 matmul (4 cy/row, no split)
    use_collective=False,  # collectives hang on this axon setup; host-side min
)


def _roles_for(n_spans, pattern):
    """Assign a role to each span; every T must be preceded by an unpaired A."""
    roles = []
    pend_a = 0
    for i in range(n_spans):
        r = pattern[i % len(pattern)]
        if r == "T" and pend_a == 0:
            r = "D"
        if r == "A":
            # an A must have a following T; if this is the last span, direct it
            if i == n_spans - 1:
                r = "D"
        if r == "A":
            pend_a += 1
        if r == "T":
            pend_a -= 1
        roles.append(r)
    # orphan A at the end shouldn't happen due to check above, but make sure
    assert pend_a == 0, roles
    return roles


def build(cfg):
    """Build the SPMD Bass program. Returns (nc, input_names)."""
    C = cfg["n_cores"]
    S = cfg["s_per_core"]
    M = cfg["m_total"]
    SPAN = cfg["span"]
    MMSZ = 512  # matmul moving free dim
    assert SPAN % MMSZ == 0 and S % SPAN == 0 and M % 128 == 0
    MT = M // 128
    NSPAN = S // SPAN
    LM_F = M // 128  # free elems per partition in [128, *] landmark layout
    roles = _roles_for(NSPAN, cfg["roles"])
    split = cfg["use_f32r_split"]

    nc = bacc.Bacc(
        "TRN2", target_bir_lowering=False, debug=False, num_devices=C
    )

    # ---- I/O ----
    spT_d = nc.dram_tensor("spT", [3, S], F32, kind="ExternalInput")
    lmT_d = nc.dram_tensor("lmT", [3, M], F32, kind="ExternalInput")
    poseT_d = nc.dram_tensor("poseT", [4, 4], F32, kind="ExternalInput")
    mmdt = F32R if split else F32  # dtype of matmul-feeding tensors
    konst_d = nc.dram_tensor("konst", [6, S], mmdt, kind="ExternalInput")  # ones/zeros
    onec_d = nc.dram_tensor("ones_col", [128, 1], F32, kind="ExternalInput")
    use_cc = cfg.get("use_collective", True)
    if use_cc:
        loss_d = nc.dram_tensor("loss", [1], F32, kind="ExternalOutput")
        nmat_d = nc.dram_tensor("nmatch", [1], I32, kind="ExternalOutput")
        mean_d = nc.dram_tensor("meand", [1], F32, kind="ExternalOutput")
    else:
        part_out_d = nc.dram_tensor("partial", [M], F32, kind="ExternalOutput")

    # round-robin issuing engines for setup DMAs -> parallel DGE queues
    _dmaq = [nc.sync, nc.gpsimd, nc.scalar]
    _dmaqi = [0]

    def dq():
        e = _dmaq[_dmaqi[0] % len(_dmaq)]
        _dmaqi[0] += 1
        return e

    with tile.TileContext(nc) as tc:
        with (
            tc.tile_pool(name="persist", bufs=1) as persist,
            tc.tile_pool(name="setup", bufs=1) as setup,
            tc.tile_pool(name="stream", bufs=3) as stream,
            tc.tile_pool(name="dram", bufs=1, space="DRAM") as dp,
        ):
            # ================= landmark features =================
            KX = 15 if split else 9
            # Engine ops need 32-aligned start partitions, so all compute stays
            # at partition base 0; cross-partition placement is done by matmul
            # outputs and DMAs only.
            #   P1 = [c, -2c, 0]   P2 = [c, 1, 0]   (both [9, M] via hom matmul)
            #   L_f32 = P1 * P2 = [c^2, -2c, 0];  rows 6-8 ones via DMA
            #   hi = f32r(L); lo = L - hi
            pt = setup.tile([4, 4], F32)
            nc.sync.dma_start(pt[:], poseT_d[:])
            lhsA = setup.tile([4, 9], F32)
            nc.vector.memset(lhsA[:], 0.0)
            nc.vector.tensor_copy(lhsA[:, 0:3], pt[:, 0:3])
            nc.vector.tensor_scalar(lhsA[:, 3:6], pt[:, 0:3], -2.0, None, ALU.mult)
            lhsB = setup.tile([4, 9], F32)
            nc.vector.memset(lhsB[:], 0.0)
            nc.vector.tensor_copy(lhsB[:, 0:3], pt[:, 0:3])
            # e3 columns (select hom ones-row): [1,1,1] into row 3, cols 3-5
            dq().dma_start(lhsB[3:4, 3:6], konst_d[0:1, 0:3].bitcast(F32))
            homT = setup.tile([4, M], F32)
            nc.sync.dma_start(homT[0:3, :], lmT_d[:])
            nc.sync.dma_start(homT[3:4, :], konst_d[0:1, 0:M].bitcast(F32))

            feat_lm_hi = persist.tile([9, M], mmdt)
            if split:
                feat_lm_x = persist.tile([KX, M], mmdt)

            with tc.tile_pool(name="lmpsum", bufs=1, space="PSUM") as lpp:
                p1 = lpp.tile([9, M], F32)
                p2 = lpp.tile([9, M], F32)
                lmb = min(MMSZ, M)
                for b in range(M // lmb):
                    sl = slice(b * lmb, (b + 1) * lmb)
                    nc.tensor.matmul(
                        p1[:, sl], lhsA[:], homT[:, sl], start=True, stop=True
                    )
                    nc.tensor.matmul(
                        p2[:, sl], lhsB[:], homT[:, sl], start=True, stop=True
                    )
                p2s = setup.tile([9, M], F32)
                nc.scalar.copy(p2s[:], p2[:])
                lmf = setup.tile([9, M], F32)
                nc.vector.tensor_mul(lmf[:], p1[:], p2s[:])
            nc.vector.tensor_copy(feat_lm_hi[0:6, :], lmf[0:6, :])
            dq().dma_start(feat_lm_hi[6:9, :], konst_d[0:3, 0:M])  # ones
            if split:
                nc.vector.tensor_sub(
                    feat_lm_x[0:6, :], lmf[0:6, :], feat_lm_hi[0:6, :].bitcast(F32)
                )
                dq().dma_start(feat_lm_x[6:9, :], konst_d[3:6, 0:M])  # zeros(L_lo 1s)
                # rows 9-11 pair with S c_lo -> L_hi(-2c); rows 12-14 pair with
                # S sq_lo -> L_hi ones
                dq().dma_start(feat_lm_x[9:12, :], feat_lm_hi[3:6, :])
                dq().dma_start(feat_lm_x[12:15, :], feat_lm_hi[6:9, :])

            # ================= splat features =================
            # feat_sp rows: 0-2 ones, 3-5 c_hi, 6-8 sq_hi, 9-11 c_lo, 12-14 sq_lo
            feat_sp = persist.tile([KX, S], mmdt)
            # nat layout [P, 256] of the flat [3*S] stream; 256 divides S so
            # DMA inner runs between [*,256] and [3, S] shapes stay commensurable
            natw = 2048
            natp = S * 3 // natw
            assert natp <= 128 and S % natw == 0
            nat = setup.tile([natp, natw], F32)
            nc.sync.dma_start(
                nat[:],
                spT_d[:].rearrange("a b -> (a b)").rearrange("(p f) -> p f", p=natp),
            )
            nat_sq = setup.tile([natp, natw], F32)
            nc.scalar.activation(nat_sq[:], nat[:], AF.Square)
            if split:
                # hi = fp32r round-on-write (matches PE operand rounding);
                # lo = x - hi, itself rounded to fp32r on write (error ~2^-24|x|)
                nat_hi = setup.tile([natp, natw], F32R)
                nc.vector.tensor_copy(nat_hi[:], nat[:])
                nat_lo = setup.tile([natp, natw], F32R)
                nc.vector.tensor_sub(nat_lo[:], nat[:], nat_hi[:].bitcast(F32))
                sq_hi = setup.tile([natp, natw], F32R)
                nc.vector.tensor_copy(sq_hi[:], nat_sq[:])
                sq_lo = setup.tile([natp, natw], F32R)
                nc.vector.tensor_sub(sq_lo[:], nat_sq[:], sq_hi[:].bitcast(F32))
            else:
                nat_hi, sq_hi = nat, nat_sq

            def row3(dst_rows, src):
                """DMA a nat-layout tile into 3 feature rows.

                Flat element order matches on both sides (partition-major)."""
                dq().dma_start(feat_sp[dst_rows : dst_rows + 3, :], src[:])

            dq().dma_start(feat_sp[0:3, :], konst_d[0:3, :])  # splat ones rows
            row3(3, nat_hi)
            row3(6, sq_hi)
            if split:
                row3(9, nat_lo)
                row3(12, sq_lo)

            # ================= main loop =================
            pp = tc.alloc_tile_pool(name="psum", bufs=4, space="PSUM")
            minsq = persist.tile([128, MT], F32)
            n_chain = roles.count("T")
            n_direct = roles.count("D")
            for mt in range(MT):
                lhs1 = feat_lm_hi[:, mt * 128 : (mt + 1) * 128]
                if split:
                    lhs2 = feat_lm_x[:, mt * 128 : (mt + 1) * 128]
                cols = setup.tile([128, n_direct + 1], F32, tag="mtcols")
                if n_chain:
                    chain = setup.tile([128, n_chain], F32, tag="mtchain")
                    scratch = setup.tile([128, SPAN], F32, tag="ttr_scratch")
                di = 0
                ti = 0
                last_a = None
                for si in range(NSPAN):
                    ps = pp.tile([128, SPAN], F32, tag="ps")
                    for h in range(SPAN // MMSZ):
                        off = si * SPAN + h * MMSZ
                        rhs1 = feat_sp[0:9, off : off + MMSZ]
                        pslice = ps[:, h * MMSZ : (h + 1) * MMSZ]
                        if split:
                            nc.tensor.matmul(
                                pslice, lhs1, rhs1, start=True, stop=False
                            )
                            rhs2 = feat_sp[0:KX, off : off + MMSZ]
                            nc.tensor.matmul(
                                pslice, lhs2, rhs2, start=False, stop=True
                            )
                        else:
                            nc.tensor.matmul(
                                pslice,
                                feat_lm_hi[:, mt * 128 : (mt + 1) * 128],
                                feat_sp[0:9, off : off + MMSZ],
                                start=True,
                                stop=True,
                            )
                    r = roles[si]
                    if r == "D":
                        nc.vector.tensor_reduce(
                            cols[:, di : di + 1], ps[:], AX.X, ALU.min
                        )
                        di += 1
                    elif r == "A":
                        sc = stream.tile([128, SPAN], F32, tag="actcopy")
                        nc.scalar.activation(sc[:], ps[:], AF.Copy)
                        last_a = sc
                    else:  # T
                        init = BIG if ti == 0 else chain[:, ti - 1 : ti]
                        nc.vector.tensor_tensor_reduce(
                            out=scratch[:],
                            in0=ps[:],
                            in1=last_a[:],
                            scale=1.0,
                            scalar=init,
                            op0=ALU.min,
                            op1=ALU.min,
                            accum_out=chain[:, ti : ti + 1],
                        )
                        ti += 1
                if ti > 0:
                    nc.vector.tensor_copy(cols[:, di : di + 1], chain[:, ti - 1 : ti])
                    di += 1
                nc.vector.tensor_reduce(
                    minsq[:, mt : mt + 1], cols[:, 0:di], AX.X, ALU.min
                )

            pp.release()

            if not use_cc:
                # per-core partial min out; global min + masked loss on host
                nc.sync.dma_start(
                    part_out_d[:].rearrange("(p f) -> p f", p=128), minsq[:]
                )
            else:
                    # ================= cross-core AllGather + min =================
                # two half-gathers: the first overlaps the second half of the loop
                MH = MT // 2
                HM = 128 * MH
                ag_outs = []
                for half in range(2):
                    part_d = dp.tile([HM], F32, tag=f"part{half}", name=f"part{half}")
                    ag_d = dp.tile(
                        [C * HM],
                        F32,
                        addr_space="Shared" if C > 4 else "Local",
                        tag=f"ag{half}",
                        name=f"ag{half}",
                    )
                    nc.sync.dma_start(
                        part_d[:].rearrange("(p f) -> p f", p=128),
                        minsq[:, half * MH : (half + 1) * MH],
                    )
                    nc.gpsimd.collective_compute(
                        "AllGather",
                        ALU.bypass,
                        replica_groups=[list(range(C))],
                        ins=[part_d[:]],
                        outs=[ag_d[:]],
                    )
                    ag_outs.append(ag_d)
                g = setup.tile([128, C * MT], F32)
                for half, ag_d in enumerate(ag_outs):
                    nc.sync.dma_start(
                        g[:, half * C * MH : (half + 1) * C * MH],
                        ag_d[:].rearrange("(r p f) -> p r f", r=C, p=128),
                    )
                gm = setup.tile([128, MT], F32)
                nc.vector.tensor_reduce(
                    gm[:].rearrange("p (h f) -> p h f", h=2),
                    g[:].rearrange("p (h r f) -> p h f r", h=2, r=C),
                    AX.X,
                    ALU.min,
                )

                # ================= replicated masked reduction =================
                msq = setup.tile([128, MT], F32)
                nc.vector.tensor_scalar(msq[:], gm[:], 0.0, None, ALU.max)
                d0 = setup.tile([128, MT], F32)
                nc.scalar.activation(d0[:], msq[:], AF.Sqrt)
                # one Newton step: d = 0.5*(d0 + msq/max(d0,eps))
                d0m = setup.tile([128, MT], F32)
                nc.vector.tensor_scalar(d0m[:], d0[:], 1e-20, None, ALU.max)
                rc = setup.tile([128, MT], F32)
                nc.vector.reciprocal(rc[:], d0m[:])
                dn = setup.tile([128, MT], F32)
                nc.vector.tensor_mul(dn[:], msq[:], rc[:])
                dd = setup.tile([128, MT], F32)
                nc.vector.tensor_add(dd[:], dn[:], d0[:])
                nc.vector.tensor_scalar(dd[:], dd[:], 0.5, None, ALU.mult)

                valid = setup.tile([128, MT], F32)
                nc.vector.tensor_scalar(valid[:], msq[:], 1.0, None, ALU.is_lt)
                vd = setup.tile([128, MT], F32)
                nc.vector.tensor_mul(vd[:], valid[:], dd[:])
                vsq = setup.tile([128, MT], F32)
                nc.vector.tensor_mul(vsq[:], valid[:], msq[:])

                stats = setup.tile([128, 3], F32)
                nc.vector.tensor_reduce(stats[:, 0:1], valid[:], AX.X, ALU.add)
                nc.vector.tensor_reduce(stats[:, 1:2], vd[:], AX.X, ALU.add)
                nc.vector.tensor_reduce(stats[:, 2:3], vsq[:], AX.X, ALU.add)

                onec = setup.tile([128, 1], F32)
                nc.sync.dma_start(onec[:], onec_d[:])
                fpp = tc.alloc_tile_pool(name="finpsum", bufs=1, space="PSUM")
                fin = fpp.tile([1, 3], F32, tag="fin", bufs=1)
                nc.tensor.matmul(fin[:], onec[:], stats[:], start=True, stop=True)

                cnt = fin[0:1, 0:1]
                den3 = setup.tile([1, 1], F32, tag="den3")
                nc.vector.tensor_scalar(den3[:], cnt, 3.0, 1.0, ALU.mult, ALU.max)
                den1 = setup.tile([1, 1], F32, tag="den1")
                nc.vector.tensor_scalar(den1[:], cnt, 1.0, None, ALU.max)
                rd3 = setup.tile([1, 1], F32, tag="rd3")
                nc.vector.reciprocal(rd3[:], den3[:])
                rd1 = setup.tile([1, 1], F32, tag="rd1")
                nc.vector.reciprocal(rd1[:], den1[:])
                loss_t = setup.tile([1, 1], F32, tag="losst")
                nc.vector.tensor_mul(loss_t[:], fin[0:1, 2:3], rd3[:])
                mean_t = setup.tile([1, 1], F32, tag="meant")
                nc.vector.tensor_mul(mean_t[:], fin[0:1, 1:2], rd1[:])
                num_i = setup.tile([1, 1], I32, tag="numi")
                nc.vector.tensor_copy(num_i[:], cnt)

                nc.sync.dma_start(loss_d[:], loss_t[:])
                nc.sync.dma_start(nmat_d[:], num_i[:])
                nc.sync.dma_start(mean_d[:], mean_t[:])
                fpp.release()

    nc.compile()
    return nc


def make_in_maps(cfg, splat_positions, camera_pose, landmarks_3d):
    C = cfg["n_cores"]
    S = cfg["s_per_core"]
    M = cfg["m_total"]
    LM_F = M // 128
    sp = np.ascontiguousarray(np.asarray(splat_positions, np.float32))
    pose = np.asarray(camera_pose, np.float32)
    lm = np.asarray(landmarks_3d, np.float32)
    konst = np.concatenate(
        [np.ones((3, S), np.float32), np.zeros((3, S), np.float32)], axis=0
    )
    ones_col = np.ones((128, 1), np.float32)
    poseT = np.ascontiguousarray(pose.T)
    lmT = np.ascontiguousarray(lm.T)
    maps = []
    for c in range(C):
        shard = sp[c * S : (c + 1) * S]
        maps.append(
            {
                "spT": np.ascontiguousarray(shard.T),
                "lmT": lmT,
                "poseT": poseT,
                "konst": konst,
                "ones_col": ones_col,
            }
        )
    return maps


_COMPILED = None


def _get_compiled():
    global _COMPILED
    if _COMPILED is None:
        _COMPILED = build(FULL_CFG)
    return _COMPILED


def kernel(
    splat_positions,
    camera_pose,
    landmarks_3d,
    landmarks_2d=None,
    camera_intrinsics=None,
    **_unused,
):
    nc = _get_compiled()
    in_maps = make_in_maps(FULL_CFG, splat_positions, camera_pose, landmarks_3d)
    core_ids = list(range(FULL_CFG["n_cores"]))
    try:
        res = run_bass_kernel_spmd(nc, in_maps, core_ids)
    except Exception:
        # one retry -- a previous run can leave the device wedged
        time.sleep(5.0)
        res = run_bass_kernel_spmd(nc, in_maps, core_ids)
    if FULL_CFG.get("use_collective", True):
        r0 = res.results[0]
        loss = np.float32(r0["loss"].reshape(-1)[0])
        num = np.int32(r0["nmatch"].reshape(-1)[0])
        meand = np.float32(r0["meand"].reshape(-1)[0])
        return loss, num, meand
    # host-side cross-core min + masked reduction (2048 elements)
    parts = np.stack([r["partial"] for r in res.results], axis=0)
    msq = np.maximum(parts.min(axis=0), np.float32(0.0)).astype(np.float32)
    d = np.sqrt(msq)
    valid = d < np.float32(1.0)
    num = np.int32(valid.sum())
    loss = np.float32(
        (msq * valid).sum(dtype=np.float32)
        / max(np.float32(3.0) * np.float32(num), np.float32(1.0))
    )
    meand = np.float32(
        (d * valid).sum(dtype=np.float32)
        / max(np.float32(num), np.float32(1.0))
    )
    return loss, num, meand


if __name__ == "__main__":
    # smoke-test build only
    build(FULL_CFG)
    print("build ok")



# revision 8
# speedup vs baseline: 2.3868x; 2.3868x over previous
"""PoseConsistencyLoss Trainium2 kernel (8-core SPMD Bass/Tile), v2.

Math: the reference's outputs (loss, num_matches, mean_distance) depend only on
the per-landmark min squared distance over all splats:
  - matched = splat_positions[argmin] makes sum(sqerr) == min_dist^2 exactly,
  - so loss = sum(valid*minsq)/max(3*num,1), mean = sum(valid*sqrt(minsq))/max(num,1),
    num = sum(minsq < 1.0).
Sharding: splats split across 8 cores (8192 each); each core computes partial
column-mins of the [8192 x 2048] distance matrix; host gathers the 8 partial
vectors, takes the global min and does the (tiny) masked reduction.

v2 speedups over the fp32 baseline (477us):
 1. The distance matmul runs in fp32r (1 cy/col vs fp32's 4): full fp32-like
    accuracy is kept by folding the hi/lo error-compensation terms into EXTRA
    CONTRACTION ROWS of a single K=21 matmul -- PE time depends only on the
    moving (splat) dim, not K, so the compensation is free:
      D2[m,n] = sum_k L[k,m]*S[k,n] with row pairs (landmark x splat):
        0-2:  hi(c^2)   x 1          3-5:  hi(-2c) x s_hi
        6-8:  lo(c^2)   x 1          9-11: lo(-2c) x s_hi
        12-14:hi(-2c)   x s_lo       15-17: 1      x hi(s^2)
        18-20: 1        x lo(s^2)
    (hi = fp32r round-on-write, lo = x - hi; dropped lo*lo term ~2^-24.)
 2. PSUM consume at 2 elem/cycle/lane on the DVE: ScalarE stages every other
    [128,2048] PSUM span to SBUF, DVE runs tensor_tensor_reduce(min,min) over
    (psum span, staged span) pairs, chaining the running min via scalar/accum.
"""

import os
import sys
import time

sys.path.insert(0, "/opt/trn_rl_repo")

import numpy as np

import concourse.bass as bass
import concourse.bacc as bacc
import concourse.tile as tile
from concourse import mybir
from concourse.bass_utils import run_bass_kernel_spmd

# Disk-cache NEFF compiles.
import concourse.bass_utils as _bu
import concourse.bass2jax as _b2j

_orig_compile_bir = _bu.compile_bir_kernel
_NEFF_CACHE = os.environ.get("BASS_NEFF_CACHE_DIR", "/tmp/bass_neff_cache")


def _cached_compile_bir(bir_json, tmpdir, neff_name="file.neff"):
    import hashlib
    import shutil

    h = hashlib.sha256(bir_json).hexdigest()[:24]
    os.makedirs(_NEFF_CACHE, exist_ok=True)
    cpath = os.path.join(_NEFF_CACHE, f"{h}_{neff_name}")
    out = os.path.join(tmpdir, neff_name)
    if os.path.exists(cpath):
        shutil.copyfile(cpath, out)
        return out
    p = _orig_compile_bir(bir_json, tmpdir, neff_name=neff_name)
    try:
        shutil.copyfile(p, cpath)
    except OSError:
        pass
    return p


_bu.compile_bir_kernel = _cached_compile_bir
_b2j.compile_bir_kernel = _cached_compile_bir

F32 = mybir.dt.float32
F32R = mybir.dt.float32r
I32 = mybir.dt.int32
AF = mybir.ActivationFunctionType
ALU = mybir.AluOpType
AX = mybir.AxisListType

BIG = 3.0e38

FULL_CFG = dict(
    n_cores=8,
    s_per_core=8192,  # splats per core
    m_total=2048,     # landmarks
    # consume modes: "ttr" = ScalarE stage + DVE tensor_tensor_reduce pairs,
    # "direct" = DVE tensor_reduce only (baseline-style, no ScalarE)
    consume="ttr",
)

K21 = 21   # contraction rows (9 base + 12 hi/lo compensation)
MMSZ = 512  # matmul moving free dim (one PSUM bank)
SPAN = 2048  # consume span (4 PSUM banks)


def build(cfg):
    C = cfg["n_cores"]
    S = cfg["s_per_core"]
    M = cfg["m_total"]
    MT = M // 128
    assert S % SPAN == 0 and SPAN % MMSZ == 0
    NSPAN = S // SPAN          # psum spans per landmark block (4)
    MM_PER_SPAN = SPAN // MMSZ  # matmuls per span (4)
    assert NSPAN % 2 == 0      # A/T pairing needs an even span count

    nc = bacc.Bacc("TRN2", target_bir_lowering=False, debug=False, num_devices=C)

    # ---- I/O ----
    spT_d = nc.dram_tensor("spT", [3, S], F32, kind="ExternalInput")
    lmT_d = nc.dram_tensor("lmT", [3, M], F32, kind="ExternalInput")
    poseT_d = nc.dram_tensor("poseT", [4, 4], F32, kind="ExternalInput")
    konst_d = nc.dram_tensor("konst", [6, S], F32, kind="ExternalInput")  # ones
    part_out_d = nc.dram_tensor("partial", [M], F32, kind="ExternalOutput")

    # round-robin issuing engines for setup DMAs -> parallel DGE queues
    _dmaq = [nc.sync, nc.gpsimd, nc.scalar]
    _dmaqi = [0]

    def dq():
        e = _dmaq[_dmaqi[0] % len(_dmaq)]
        _dmaqi[0] += 1
        return e

    with tile.TileContext(nc) as tc:
        with (
            tc.tile_pool(name="persist", bufs=1) as persist,
            tc.tile_pool(name="setup", bufs=1) as setup,
            tc.tile_pool(name="stage", bufs=2) as stage,
        ):
            # ================= splat features (rhs) =================
            # nat layout: flat [3*S] -> [96, 256]; cols 256:512 hold squares.
            natp, natw = 96, S * 3 // 96
            nat2 = setup.tile([natp, 2 * natw], F32)
            nc.sync.dma_start(
                nat2[:, 0:natw],
                spT_d[:].rearrange("a b -> (a b)").rearrange("(p f) -> p f", p=natp),
            )
            nc.scalar.activation(nat2[:, natw : 2 * natw], nat2[:, 0:natw], AF.Square)
            nat2_hi = setup.tile([natp, 2 * natw], F32R)
            nc.vector.tensor_copy(nat2_hi[:], nat2[:])
            nat2_lo = setup.tile([natp, 2 * natw], F32R)
            nc.vector.tensor_sub(nat2_lo[:], nat2[:], nat2_hi[:].bitcast(F32))

            # feat_sp rows: 0-2 ones, 3-5 s_hi, 6-8 ones, 9-11 s_hi,
            #               12-14 s_lo, 15-17 sq_hi, 18-20 sq_lo
            feat_sp = persist.tile([K21, S], F32R)

            # dma_start matches flat partition-major element order, so a
            # [96, 256] source lands in [3, 8192] rows directly.
            dq().dma_start(feat_sp[0:3, :], konst_d[0:3, :].bitcast(F32R))
            dq().dma_start(feat_sp[6:9, :], konst_d[3:6, :].bitcast(F32R))
            dq().dma_start(feat_sp[3:6, :], nat2_hi[:, 0:natw])
            dq().dma_start(feat_sp[9:12, :], nat2_hi[:, 0:natw])
            dq().dma_start(feat_sp[12:15, :], nat2_lo[:, 0:natw])
            dq().dma_start(feat_sp[15:18, :], nat2_hi[:, natw : 2 * natw])
            dq().dma_start(feat_sp[18:21, :], nat2_lo[:, natw : 2 * natw])

            # ================= landmark features (lhsT) =================
            pt = setup.tile([4, 4], F32)
            nc.sync.dma_start(pt[:], poseT_d[:])
            homT = setup.tile([4, M], F32)
            nc.sync.dma_start(homT[0:3, :], lmT_d[:])
            nc.sync.dma_start(homT[3:4, :], konst_d[0:1, 0:M])

            # feat_lm rows: 0-2 hi(c^2), 3-5 hi(-2c), 6-8 lo(c^2),
            #               9-11 lo(-2c), 12-14 hi(-2c), 15-20 ones
            # (engine ops must start at partition 0, so hi/lo are computed in
            #  base-0 packed tiles and DMA'd into their feat_lm rows)
            feat_lm = persist.tile([K21, M], F32R)
            pk = setup.tile([6, M], F32)  # rows 0-2 c^2, rows 3-5 -2c
            pm2 = setup.tile([3, M], F32)
            with tc.tile_pool(name="lmpsum", bufs=1, space="PSUM") as lpp:
                cam = lpp.tile([4, M], F32)
                for b in range(M // MMSZ):
                    sl = slice(b * MMSZ, (b + 1) * MMSZ)
                    nc.tensor.matmul(cam[:, sl], pt[:], homT[:, sl], start=True, stop=True)
                nc.scalar.activation(pk[0:3, :], cam[0:3, :], AF.Square)
                nc.scalar.activation(pm2[:], cam[0:3, :], AF.Copy, scale=-2.0)
            nc.sync.dma_start(pk[3:6, :], pm2[:])
            pk_hi = setup.tile([6, M], F32R)
            nc.vector.tensor_copy(pk_hi[:], pk[:])
            pk_lo = setup.tile([6, M], F32R)
            nc.vector.tensor_sub(pk_lo[:], pk[:], pk_hi[:].bitcast(F32))
            dq().dma_start(feat_lm[0:6, :], pk_hi[:])
            dq().dma_start(feat_lm[6:12, :], pk_lo[:])
            dq().dma_start(feat_lm[12:15, :], pk_hi[3:6, :])
            dq().dma_start(feat_lm[15:21, :], konst_d[0:6, 0:M].bitcast(F32R))

            # ================= main loop =================
            pp = tc.alloc_tile_pool(name="psum", bufs=2, space="PSUM")
            minsq = persist.tile([128, MT], F32)
            chain = persist.tile([128, MT], F32)
            use_ttr = cfg.get("consume", "ttr") == "ttr"

            def span_mms(ps, mt, si):
                lhs = feat_lm[:, mt * 128 : (mt + 1) * 128]
                for h in range(MM_PER_SPAN):
                    off = si * SPAN + h * MMSZ
                    nc.tensor.matmul(
                        ps[:, h * MMSZ : (h + 1) * MMSZ],
                        lhs,
                        feat_sp[:, off : off + MMSZ],
                        start=True,
                        stop=True,
                    )

            mode = cfg.get("consume", "ttr")
            for mt in range(MT):
                if mode == "stageonly":
                    # ACT stages span A (result unused); DVE direct-reduces both
                    cols = setup.tile([128, NSPAN], F32, tag="mtcols")
                    for si in range(NSPAN):
                        ps = pp.tile([128, SPAN], F32, tag="ps")
                        span_mms(ps, mt, si)
                        if si % 2 == 0:
                            stg = stage.tile([128, SPAN], F32, tag="stg")
                            nc.scalar.activation(stg[:], ps[:], AF.Copy)
                        nc.vector.tensor_reduce(
                            cols[:, si : si + 1], ps[:], AX.X, ALU.min
                        )
                    nc.vector.tensor_reduce(
                        minsq[:, mt : mt + 1], cols[:], AX.X, ALU.min
                    )
                elif mode == "ttrsbuf":
                    # ACT stages BOTH spans; TTR runs SBUF x SBUF
                    for half in range(NSPAN // 2):
                        psA = pp.tile([128, SPAN], F32, tag="ps")
                        span_mms(psA, mt, 2 * half)
                        stg = stage.tile([128, SPAN], F32, tag="stg")
                        nc.scalar.activation(stg[:], psA[:], AF.Copy)
                        psB = pp.tile([128, SPAN], F32, tag="ps")
                        span_mms(psB, mt, 2 * half + 1)
                        stg2 = stage.tile([128, SPAN], F32, tag="stg2")
                        nc.scalar.activation(stg2[:], psB[:], AF.Copy)
                        scr = stage.tile([128, SPAN], F32, tag="scr")
                        first = half == 0
                        last = half == NSPAN // 2 - 1
                        nc.vector.tensor_tensor_reduce(
                            out=scr[:],
                            in0=stg2[:],
                            in1=stg[:],
                            scale=1.0,
                            scalar=BIG if first else chain[:, mt : mt + 1],
                            op0=ALU.min,
                            op1=ALU.min,
                            accum_out=(
                                minsq[:, mt : mt + 1] if last else chain[:, mt : mt + 1]
                            ),
                        )
                elif use_ttr:
                    for half in range(NSPAN // 2):
                        # span A: matmuls then ScalarE copy PSUM->SBUF
                        psA = pp.tile([128, SPAN], F32, tag="ps")
                        span_mms(psA, mt, 2 * half)
                        stg = stage.tile([128, SPAN], F32, tag="stg")
                        nc.scalar.activation(stg[:], psA[:], AF.Copy)
                        # span B: matmuls then DVE TTR over (psum, staged) pair
                        psB = pp.tile([128, SPAN], F32, tag="ps")
                        span_mms(psB, mt, 2 * half + 1)
                        scr = stage.tile([128, SPAN], F32, tag="scr")
                        first = half == 0
                        last = half == NSPAN // 2 - 1
                        nc.vector.tensor_tensor_reduce(
                            out=scr[:],
                            in0=psB[:],
                            in1=stg[:],
                            scale=1.0,
                            scalar=BIG if first else chain[:, mt : mt + 1],
                            op0=ALU.min,
                            op1=ALU.min,
                            accum_out=(
                                minsq[:, mt : mt + 1] if last else chain[:, mt : mt + 1]
                            ),
                        )
                else:
                    cols = setup.tile([128, NSPAN], F32, tag="mtcols")
                    for si in range(NSPAN):
                        ps = pp.tile([128, SPAN], F32, tag="ps")
                        span_mms(ps, mt, si)
                        nc.vector.tensor_reduce(
                            cols[:, si : si + 1], ps[:], AX.X, ALU.min
                        )
                    nc.vector.tensor_reduce(
                        minsq[:, mt : mt + 1], cols[:], AX.X, ALU.min
                    )
            pp.release()

            # per-core partial min out; global min + masked loss on host
            nc.sync.dma_start(
                part_out_d[:].rearrange("(p f) -> p f", p=128), minsq[:]
            )

    nc.compile()
    return nc


def make_in_maps(cfg, splat_positions, camera_pose, landmarks_3d):
    C = cfg["n_cores"]
    S = cfg["s_per_core"]
    sp = np.ascontiguousarray(np.asarray(splat_positions, np.float32))
    pose = np.asarray(camera_pose, np.float32)
    lm = np.asarray(landmarks_3d, np.float32)
    konst = np.ones((6, S), np.float32)
    poseT = np.ascontiguousarray(pose.T)
    lmT = np.ascontiguousarray(lm.T)
    maps = []
    for c in range(C):
        shard = sp[c * S : (c + 1) * S]
        maps.append(
            {
                "spT": np.ascontiguousarray(shard.T),
                "lmT": lmT,
                "poseT": poseT,
                "konst": konst,
            }
        )
    return maps


_COMPILED = None


def _get_compiled():
    global _COMPILED
    if _COMPILED is None:
        _COMPILED = build(FULL_CFG)
    return _COMPILED


def kernel(
    splat_positions,
    camera_pose,
    landmarks_3d,
    landmarks_2d=None,
    camera_intrinsics=None,
    **_unused,
):
    nc = _get_compiled()
    in_maps = make_in_maps(FULL_CFG, splat_positions, camera_pose, landmarks_3d)
    core_ids = list(range(FULL_CFG["n_cores"]))
    try:
        res = run_bass_kernel_spmd(nc, in_maps, core_ids)
    except Exception:
        # one retry -- a previous run can leave the device wedged
        time.sleep(5.0)
        res = run_bass_kernel_spmd(nc, in_maps, core_ids)
    # host-side cross-core min + masked reduction (2048 elements)
    parts = np.stack([r["partial"] for r in res.results], axis=0)
    msq = np.maximum(parts.min(axis=0), np.float32(0.0)).astype(np.float32)
    d = np.sqrt(msq)
    valid = d < np.float32(1.0)
    num = np.int32(valid.sum())
    loss = np.float32(
        (msq * valid).sum(dtype=np.float32)
        / max(np.float32(3.0) * np.float32(num), np.float32(1.0))
    )
    meand = np.float32(
        (d * valid).sum(dtype=np.float32) / max(np.float32(num), np.float32(1.0))
    )
    return loss, num, meand


if __name__ == "__main__":
    build(FULL_CFG)
    print("build ok")
